# revision 55
# baseline (speedup 1.0000x reference)
"""Causal multi-head attention (b=2, n=2048, d=768, 12 heads) on 8 TRN2 NeuronCores.

Sharding: batch x head-group. Core c handles batch c//4 and heads 3*(c%4) .. 3*(c%4)+2.
Each core gets xT = x[b].T plus W.T column slices for its 3 heads, computes the
unnormalized attention output (transposed) plus softmax denominators; the host
divides, transposes, and concatenates slabs into the full [2, 2048, 768].

Per-core algorithm (everything transposed so softmax reductions ride on matmuls):
  qT/kT/vT = (W.T slice).T @ xT            TensorE, per 512-col span
  v_nat[j, m] = transpose(vT) + ones column -> stationary [128, 65] per j-tile
  per head, per 512-col i-span:
    sT[j, i] = kT_h[:, jtile].T @ qT[:, span]   (psum, causally skipped/sliced)
    p = exp(sT) unshifted (max causal score ~66 fits fp32), bf16; diagonal
        128-blocks multiplied by a 0/1 bf16 triangular mask
    av[0:65, span] += v_nat[jtile].T @ p    (row 64 accumulates sum(p) = denom)
  av -> DRAM; host computes (av[0:64]/av[64]).T per head.

Perf facts measured on this hardware (see also the HAM/tile_position notes):
  - PSUM bank = 512 fp32; matmul outputs stay within one bank
  - keep K=128 and a single 128x128 PE mode everywhere: 64x128 row-tiled pairs
    DO run concurrently but their LDWEIGHTS cannot hide behind same-row-group
    in-flight MMs (~175ns exposed per wall) and the mode mixing throttles the
    HAM clock gate to 1.2 GHz -- measured net LOSS vs plain 128-mode
  - f32r 1.06 cyc/row @2.4GHz warm; ~165ns fixed per MM (~58ns exposed b2b)
  - f32r identity transposes run ~281ns vs ~378ns for fp32 (4-pass)
  - ACT exp = 0.84ns/col + ~250ns/instr and is the attention-phase co-bottleneck:
    span s+1's projection work is interleaved (evenly spread) into span s's
    attention stream at the post-exp insertion point, which is always safe for
    the tile-ring WAR tracking (every live sc tile's reader is already emitted)
  - DVE TensorTensor cannot touch PSUM (BIR verifier); masks ride bf16 SBUF
"""
import sys

if "/opt/trn_rl_repo" not in sys.path:
    sys.path.insert(0, "/opt/trn_rl_repo")

from contextlib import ExitStack

import numpy as np

import concourse.bass as bass
import concourse.tile as tile
from concourse import bacc, mybir, bass_utils
from concourse.masks import make_identity

F32 = mybir.dt.float32
F32R = mybir.dt.float32r
BF16 = mybir.dt.bfloat16

P = 128
H = 64
SPAN = 512
HD = 64

B, N, D, NH = 2, 2048, 768, 12
HL = 3                       # heads per core
DL = HL * HD                 # 192
N_CORES = 8
KT = D // P                  # 6 contraction chunks
KH = KT // 2                 # kt per x/w half
NS = N // SPAN               # 4 spans
NT = N // P                  # 16 j-tiles
CPS = SPAN // P              # 4 chunks per span

DT_PROJ = F32R               # x, W, qT/kT/vT
DT_P = BF16                  # p = exp(scores), v_nat
WARMUP_N = 8                 # before first projection (cover to x00a ~13.5us)
WARMUP_MID = 3               # between kt halves of chunk 0 (cover to x00b ~15.6us)
EXP = mybir.ActivationFunctionType.Exp


def _build(nc, tc, dt_proj, dt_p):
    # host pre-packs: xt[p, (ns, kth, ktl, c)], wc[p, (kt, m)] with
    # m = packed weight columns [q01 | k01 | v01 | k2+v2 | q2]
    xt = nc.dram_tensor("xt", [P, N * KT], dt_proj, kind="ExternalInput").ap()
    wc = nc.dram_tensor("wc", [P, KT * 3 * DL], dt_proj,
                        kind="ExternalInput").ap()
    o = nc.dram_tensor("o", [HL * (HD + 1), N], F32, kind="ExternalOutput").ap()

    with ExitStack() as ctx:
        pool = lambda name, bufs, **kw: ctx.enter_context(
            tc.tile_pool(name=name, bufs=bufs, **kw))
        const_pool = pool("const", 1)
        xpool = pool("x", 2 * NS)
        wpool = pool("w", 2)
        qk_pool = pool("qk", NS)
        kz_pool = pool("kz", HL * NS)
        vnat_pool = pool("vnat", 1)
        ppool = pool("p", 6)
        osb_pool = pool("osb", 3)
        ps = pool("ps", 2, space="PSUM")        # [128,1024] sc pair tiles: 2x2 banks
        ps_pj = pool("ps_pj", 2, space="PSUM")  # [128,512] proj/transpose: 2x1 bank
        ps_av = pool("ps_av", 2, space="PSUM")  # [65,512] accumulators: 2x1 bank

        ident = const_pool.tile([P, P], F32)
        make_identity(nc, ident[:])
        ident_r = const_pool.tile([P, P], dt_proj)
        nc.vector.tensor_copy(ident_r[:], ident[:])
        # multiplicative causal mask for [key-partition, query-col] diag blocks:
        # 1 where key <= query, 0 where key > query (bf16, post-exp multiply)
        tri16 = const_pool.tile([P, P], dt_p)
        nc.gpsimd.memset(tri16[:], 0.0)
        nc.gpsimd.affine_select(
            out=tri16[:], in_=tri16[:], compare_op=mybir.AluOpType.is_gt,
            fill=1.0, base=0, pattern=[[-1, P]], channel_multiplier=1)
        ones32 = const_pool.tile([P, 2 * NT], F32)
        nc.gpsimd.memset(ones32[:], 1.0)
        zeros = const_pool.tile([P, SPAN], F32)
        nc.gpsimd.memset(zeros[:], 0.0)
        zeros_r = const_pool.tile([P, SPAN], dt_proj)
        nc.vector.tensor_copy(zeros_r[:], zeros[:])

        # ---- DMA inputs: w halves + 8 x chunks (span, kt-half) ----
        x_tiles = [xpool.tile([P, KH * SPAN], dt_proj, tag="x", name=f"x{i}")
                   for i in range(2 * NS)]

        def x_slice(ns, kt):
            t = x_tiles[2 * ns + kt // KH]
            b = (kt % KH) * SPAN
            return t[:, b:b + SPAN]

        def dma_x(ns, half):
            w = KH * SPAN
            i = 2 * ns + half
            nc.sync.dma_start(x_tiles[i][:], xt[:, i * w:(i + 1) * w])

        # weights packed per m-chunk: wc columns = (chunk, kt, m) so chunk 0
        # only gates on its own 3KB slice (ready ~11us) and the x halves
        CH_W = (P, P, P, P, HD)
        CH_OFF = [sum(CH_W[:i]) for i in range(len(CH_W) + 1)]
        w_ch = [wpool.tile([P, KT * CH_W[ci]], dt_proj, tag=f"w{ci}",
                           name=f"w{ci}") for ci in range(5)]

        def w_slice(ci, kt):
            msz = CH_W[ci]
            return w_ch[ci][:, kt * msz:(kt + 1) * msz]

        def dma_w(ci):
            b = KT * CH_OFF[ci]
            w = KT * CH_W[ci]
            nc.sync.dma_start(w_ch[ci][:], wc[:, b:b + w])

        dma_w(0)
        dma_x(0, 0)
        dma_x(0, 1)
        for ci in range(1, 5):
            dma_w(ci)
        for ns in range(1, NS):
            dma_x(ns, 0)
            dma_x(ns, 1)

        # ---- per-span q/k/v tiles (zero-padded K=128 layout) ----
        qT01 = [qk_pool.tile([P, SPAN], dt_proj, tag="q01", name=f"q01_{i}") for i in range(NS)]
        qT2z = [qk_pool.tile([P, SPAN], dt_proj, tag="q2z", name=f"q2z_{i}") for i in range(NS)]
        vT01 = [qk_pool.tile([P, SPAN], dt_proj, tag="v01", name=f"v01_{i}") for i in range(NS)]
        vT2z = [qk_pool.tile([P, SPAN], dt_proj, tag="v2z", name=f"v2z_{i}") for i in range(NS)]
        kTz = [[kz_pool.tile([P, SPAN], dt_proj, tag="kz", name=f"kz_{h}_{i}")
                for i in range(NS)] for h in range(HL)]

        def zfill(ap):
            nc.vector.tensor_copy(ap, zeros[0:ap.shape[0], 0:ap.shape[1]])

        for ns in range(NS):
            zfill(qT2z[ns][HD:P, :])
            zfill(vT2z[ns][0:HD, :])
            zfill(kTz[0][ns][HD:P, :])
            zfill(kTz[1][ns][0:HD, :])
            zfill(kTz[2][ns][HD:P, :])

        # v natural layout: heads 0,1 interleaved per j-tile [v0|1|v1|1], head 2
        # separate [v2|1]; the ones column accumulates the softmax denominator.
        v_nat01 = vnat_pool.tile([P, NT * 2 * (HD + 1)], dt_p, tag="vnat01")
        v_nat2 = vnat_pool.tile([P, NT * (HD + 1)], dt_p, tag="vnat2")
        c01 = v_nat01[:].rearrange("p (t c) -> p t c", c=HD + 1)[:, :, HD]
        c2 = v_nat2[:].rearrange("p (t c) -> p t c", c=HD + 1)[:, :, HD]
        nc.vector.tensor_copy(c01, ones32[:])
        nc.vector.tensor_copy(c2, ones32[:, 0:NT])

        def vnat(h, jt):
            if h < 2:
                b = jt * 2 * (HD + 1) + h * (HD + 1)
                return v_nat01[:, b:b + HD + 1]
            b = jt * (HD + 1)
            return v_nat2[:, b:b + HD + 1]

        # ---- warmup: keep the PE busy while the first DMAs land ----
        warm = ps_pj.tile([P, SPAN], F32, tag="ps_pj", name="warm")

        def warmup(n):
            for _ in range(n):
                nc.tensor.matmul(warm[:], ident_r[:], zeros_r[:],
                                 start=True, stop=True)

        warmup(WARMUP_N)

        # ---- projections as an op list (5 chunk ops + 8 transpose ops) ----
        m_chunks = ((0, P, "q01"), (1, P, "k01"), (2, P, "v01"),
                    (3, P, "k2v2"), (4, HD, "q2"))

        def chunk_op(ns, ci, msz, what, midfill=0):
            pt = ps_pj.tile([msz, SPAN], F32, tag="ps_pj", name=f"pj_{ns}_{what}")
            for kt in range(KT):
                if midfill and kt == KH:
                    warmup(midfill)
                nc.tensor.matmul(
                    pt[:], w_slice(ci, kt), x_slice(ns, kt),
                    start=(kt == 0), stop=(kt == KT - 1))
            if what == "q01":
                nc.vector.tensor_copy(qT01[ns][:], pt[:])
            elif what == "k01":
                nc.vector.tensor_copy(kTz[0][ns][0:HD, :], pt[0:HD, :])
                nc.vector.tensor_copy(kTz[1][ns][HD:P, :], pt[HD:P, :])
            elif what == "v01":
                nc.vector.tensor_copy(vT01[ns][:], pt[:])
            elif what == "k2v2":
                nc.vector.tensor_copy(kTz[2][ns][0:HD, :], pt[0:HD, :])
                nc.vector.tensor_copy(vT2z[ns][HD:P, :], pt[HD:P, :])
            else:
                nc.vector.tensor_copy(qT2z[ns][0:HD, :], pt[:])

        def transp01_op(ns, c):
            jt = ns * CPS + c
            tp = ps_pj.tile([P, P], dt_proj, tag="ps_pj", name=f"tp_{jt}")
            nc.tensor.transpose(tp[:], vT01[ns][:, c * P:(c + 1) * P],
                                ident_r[:])
            nc.vector.tensor_copy(
                v_nat01[:].rearrange("p (t c) -> p t c", c=HD + 1)[
                    :, 2 * jt:2 * jt + 2, 0:HD],
                tp[:].rearrange("p (t c) -> p t c", c=HD))

        def transp2_op(ns, c):
            jt = ns * CPS + c
            tp2 = ps_pj.tile([P, P], dt_proj, tag="ps_pj", name=f"tp2_{jt}")
            nc.tensor.transpose(tp2[:], vT2z[ns][:, c * P:(c + 1) * P],
                                ident_r[:])
            nc.vector.tensor_copy(
                v_nat2[:, jt * (HD + 1):jt * (HD + 1) + HD], tp2[:, HD:P])

        def proj_ops(ns):
            ops = [lambda a=ci, b=msz, w=what: chunk_op(ns, a, b, w)
                   for (ci, msz, what) in m_chunks]
            for c in range(CPS):
                ops.append(lambda c=c: transp01_op(ns, c))
                ops.append(lambda c=c: transp2_op(ns, c))
            return ops

        pending = []
        pavs_left = [1]

        def drain_even():
            # spread pending ops evenly over the remaining insertion points
            if not pending:
                return False
            k = -(-len(pending) // max(pavs_left[0], 1))
            for _ in range(k):
                if pending:
                    pending.pop(0)()
            return True

        def finalize(s, h, av):
            ob = osb_pool.tile([HD + 1, SPAN], F32, tag="osb", name=f"ob{s}_{h}")
            nc.vector.tensor_copy(ob[:], av[:])
            nc.sync.dma_start(
                o[h * (HD + 1):(h + 1) * (HD + 1),
                  s * SPAN:(s + 1) * SPAN], ob[:])

        # ---- attention: heads 0,1 fused pair loop; head 2 solo ----
        def attn01(s):
            njt = CPS * (s + 1)
            av0 = ps_av.tile([HD + 1, SPAN], F32, tag="ps_av", name=f"av0_{s}")
            av1 = ps_av.tile([HD + 1, SPAN], F32, tag="ps_av", name=f"av1_{s}")
            live = {}

            def emit_sc(jt):
                c_d = jt - CPS * s
                n0 = max(c_d, 0) * P
                ns_k, ck = jt // CPS, jt % CPS
                sc = ps.tile([P, 2 * SPAN], F32, tag="ps", name=f"sc01_{s}_{jt}")
                nc.tensor.matmul(sc[:, n0:SPAN],
                                 kTz[0][ns_k][:, ck * P:(ck + 1) * P],
                                 qT01[s][:, n0:SPAN], start=True, stop=True)
                nc.tensor.matmul(sc[:, SPAN + n0:2 * SPAN],
                                 kTz[1][ns_k][:, ck * P:(ck + 1) * P],
                                 qT01[s][:, n0:SPAN], start=True, stop=True)
                live[jt] = (sc, n0, c_d >= 0)

            def emit_pav(jt):
                sc, n0, diag = live.pop(jt)
                p = ppool.tile([P, 2 * SPAN], dt_p, tag="p", name=f"p01_{s}_{jt}")
                sc3 = sc[:].rearrange("q (t c) -> q t c", c=SPAN)
                p3 = p[:].rearrange("q (t c) -> q t c", c=SPAN)
                nc.scalar.activation(p3[:, :, n0:SPAN], sc3[:, :, n0:SPAN], EXP)
                if diag:
                    nc.vector.tensor_mul(p[:, n0:n0 + P], p[:, n0:n0 + P],
                                         tri16[:])
                    nc.vector.tensor_mul(
                        p[:, SPAN + n0:SPAN + n0 + P],
                        p[:, SPAN + n0:SPAN + n0 + P], tri16[:])
                # safe insertion point: every live sc tile's reader is emitted;
                # proj bursts and the two-ahead sc keep Tensor fed through the
                # exp latency
                drain_even()
                if jt + 2 < njt and jt + 2 not in live:
                    emit_sc(jt + 2)
                pavs_left[0] -= 1
                st, sp = (jt == 0), (jt == njt - 1)
                nc.tensor.matmul(av0[:, n0:SPAN], vnat(0, jt), p[:, n0:SPAN],
                                 start=st, stop=sp)
                nc.tensor.matmul(av1[:, n0:SPAN], vnat(1, jt),
                                 p[:, SPAN + n0:2 * SPAN], start=st, stop=sp)

            emit_sc(0)
            if njt > 1:
                emit_sc(1)
            for jt in range(njt):
                emit_pav(jt)
            finalize(s, 0, av0)
            finalize(s, 1, av1)

        def attn2(s):
            njt = CPS * (s + 1)
            av2 = ps_av.tile([HD + 1, SPAN], F32, tag="ps_av", name=f"av2_{s}")
            live = {}

            def emit_sc(jt):
                c_d = jt - CPS * s
                n0 = max(c_d, 0) * P
                ns_k, ck = jt // CPS, jt % CPS
                sc = ps.tile([P, 2 * SPAN], F32, tag="ps", name=f"sc2_{s}_{jt}")
                nc.tensor.matmul(sc[:, n0:SPAN],
                                 kTz[2][ns_k][:, ck * P:(ck + 1) * P],
                                 qT2z[s][:, n0:SPAN], start=True, stop=True)
                live[jt] = (sc, n0, c_d >= 0)

            def emit_pav(jt):
                sc, n0, diag = live.pop(jt)
                p = ppool.tile([P, 2 * SPAN], dt_p, tag="p", name=f"p2_{s}_{jt}")
                nc.scalar.activation(p[:, n0:SPAN], sc[:, n0:SPAN], EXP)
                if diag:
                    nc.vector.tensor_mul(p[:, n0:n0 + P], p[:, n0:n0 + P],
                                         tri16[:])
                drain_even()
                if jt + 2 < njt and jt + 2 not in live:
                    emit_sc(jt + 2)
                pavs_left[0] -= 1
                nc.tensor.matmul(av2[:, n0:SPAN], vnat(2, jt), p[:, n0:SPAN],
                                 start=(jt == 0), stop=(jt == njt - 1))

            emit_sc(0)
            if njt > 1:
                emit_sc(1)
            for jt in range(njt):
                emit_pav(jt)
            finalize(s, 2, av2)

        # spans 0 and 1 project standalone (warmup mid-fill covers the
        # kt0-2 -> kt3-5 x-DMA boundary of the first chunk); spans 2 and 3
        # interleave into the attention streams of spans 0 and 1, which are
        # small and latency-chained — the proj bursts keep the PE duty cycle
        # high so the HAM clock stays at 2.4 GHz
        for ns0 in range(2):
            first = ns0 == 0
            for (ci, msz, what) in m_chunks:
                chunk_op(ns0, ci, msz, what,
                         midfill=WARMUP_MID if first else 0)
                first = False
            for c in range(CPS):
                transp01_op(ns0, c)
                transp2_op(ns0, c)
        for s in range(NS):
            pending.extend(proj_ops(s + 2) if s + 2 < NS else [])
            pavs_left[0] = 2 * CPS * (s + 1)
            attn01(s)
            attn2(s)
            while pending:
                pending.pop(0)()


_NC_CACHE = {}


def _get_module(dt_proj=DT_PROJ, dt_p=DT_P):
    key = (dt_proj, dt_p)
    if key not in _NC_CACHE:
        nc = bacc.Bacc("TRN2", target_bir_lowering=False, debug=False)
        with tile.TileContext(nc) as tc:
            _build(nc, tc, dt_proj, dt_p)
        nc.compile()
        _NC_CACHE[key] = nc
    return _NC_CACHE[key]


def _in_maps(x, Wq, Wk, Wv):
    maps = []
    xT = [np.ascontiguousarray(
        x[b].T.reshape(KT, P, NS, SPAN).transpose(1, 2, 0, 3).reshape(P, -1))
        for b in range(B)]
    WqT, WkT, WvT = Wq.T, Wk.T, Wv.T
    for c in range(N_CORES):
        bc, g = divmod(c, N_CORES // B)
        s0 = g * DL
        wcomb = np.concatenate([
            WqT[:, s0:s0 + P], WkT[:, s0:s0 + P], WvT[:, s0:s0 + P],
            WkT[:, s0 + P:s0 + DL], WvT[:, s0 + P:s0 + DL],
            WqT[:, s0 + P:s0 + DL]], axis=1)
        # pack as (chunk, kt, m): per m-chunk, kt-major
        w3 = wcomb.reshape(KT, P, 3 * DL).transpose(1, 0, 2)  # [P, kt, m]
        parts = []
        for c0, c1 in ((0, P), (P, 2 * P), (2 * P, 3 * P), (3 * P, 4 * P),
                       (4 * P, 4 * P + HD)):
            parts.append(w3[:, :, c0:c1].reshape(P, -1))
        wpk = np.ascontiguousarray(np.concatenate(parts, axis=1))
        maps.append({
            "xt": xT[bc],
            "wc": wpk,
        })
    return maps


def kernel(x, Wq, Wk, Wv, _trace=False, _tmpdir=None, **_kw):
    x = np.asarray(x, dtype=np.float32)
    Wq = np.asarray(Wq, dtype=np.float32)
    Wk = np.asarray(Wk, dtype=np.float32)
    Wv = np.asarray(Wv, dtype=np.float32)
    assert x.shape == (B, N, D) and Wq.shape == (D, D)

    nc = _get_module()
    res = bass_utils.run_bass_kernel_spmd(
        nc, _in_maps(x, Wq, Wk, Wv), core_ids=list(range(N_CORES)),
        trace=_trace, tmpdir=_tmpdir)
    out = np.empty((B, N, D), np.float32)
    for c in range(N_CORES):
        bc, g = divmod(c, N_CORES // B)
        oT = res.results[c]["o"].astype(np.float64)
        for h in range(HL):
            blk = oT[h * (HD + 1):h * (HD + 1) + HD, :]
            den = oT[h * (HD + 1) + HD, :]
            out[bc, :, g * DL + h * HD:g * DL + (h + 1) * HD] = \
                (blk / den).T.astype(np.float32)
    if _trace:
        return out, res
    return out


# revision 56
# speedup vs baseline: 21541.8975x; 21541.8975x over previous
"""Causal multi-head attention (b=2, n=2048, d=768, 12 heads) on 8 TRN2 NeuronCores.

Sharding: batch x head-group. Core c handles batch c//4 and heads 3*(c%4) .. 3*(c%4)+2.
Each core gets xT = x[b].T plus W.T column slices for its 3 heads, computes the
unnormalized attention output (transposed) plus softmax denominators; the host
divides, transposes, and concatenates slabs into the full [2, 2048, 768].

Per-core algorithm (everything transposed so softmax reductions ride on matmuls):
  qT/kT/vT = (W.T slice).T @ xT            TensorE, per 512-col span
  v_nat[j, m] = transpose(vT) + ones column -> stationary [128, 65] per j-tile
  per head, per 512-col i-span:
    sT[j, i] = kT_h[:, jtile].T @ qT[:, span]   (psum, causally skipped/sliced)
    p = exp(sT) unshifted (max causal score ~66 fits fp32), bf16; diagonal
        128-blocks multiplied by a 0/1 bf16 triangular mask
    av[0:65, span] += v_nat[jtile].T @ p    (row 64 accumulates sum(p) = denom)
  av -> DRAM; host computes (av[0:64]/av[64]).T per head.

Perf facts measured on this hardware (see also the HAM/tile_position notes):
  - PSUM bank = 512 fp32; matmul outputs stay within one bank
  - keep K=128 and a single 128x128 PE mode everywhere: 64x128 row-tiled pairs
    DO run concurrently but their LDWEIGHTS cannot hide behind same-row-group
    in-flight MMs (~175ns exposed per wall) and the mode mixing throttles the
    HAM clock gate to 1.2 GHz -- measured net LOSS vs plain 128-mode
  - f32r 1.06 cyc/row @2.4GHz warm; ~165ns fixed per MM (~58ns exposed b2b)
  - f32r identity transposes run ~281ns vs ~378ns for fp32 (4-pass)
  - ACT exp = 0.84ns/col + ~250ns/instr and is the attention-phase co-bottleneck:
    span s+1's projection work is interleaved (evenly spread) into span s's
    attention stream at the post-exp insertion point, which is always safe for
    the tile-ring WAR tracking (every live sc tile's reader is already emitted)
  - DVE TensorTensor cannot touch PSUM (BIR verifier); masks ride bf16 SBUF
"""
import sys

if "/opt/trn_rl_repo" not in sys.path:
    sys.path.insert(0, "/opt/trn_rl_repo")

from contextlib import ExitStack

import numpy as np

import concourse.bass as bass
import concourse.tile as tile
from concourse import bacc, mybir, bass_utils
from concourse.masks import make_identity

F32 = mybir.dt.float32
F32R = mybir.dt.float32r
BF16 = mybir.dt.bfloat16

P = 128
H = 64
SPAN = 512
HD = 64

B, N, D, NH = 2, 2048, 768, 12
HL = 3                       # heads per core
DL = HL * HD                 # 192
N_CORES = 8
KT = D // P                  # 6 contraction chunks
KH = KT // 2                 # kt per x/w half
NS = N // SPAN               # 4 spans
NT = N // P                  # 16 j-tiles
CPS = SPAN // P              # 4 chunks per span

DT_PROJ = F32R               # x, W, qT/kT/vT
DT_P = BF16                  # p = exp(scores), v_nat
WARMUP_N = 8                 # before first projection (cover to x00a ~13.5us)
WARMUP_MID = 3               # between kt halves of chunk 0 (cover to x00b ~15.6us)
EXP = mybir.ActivationFunctionType.Exp


def _build(nc, tc, dt_proj, dt_p):
    # host pre-packs: xt[p, (ns, kth, ktl, c)], wc[p, (kt, m)] with
    # m = packed weight columns [q01 | k01 | v01 | k2+v2 | q2]
    xt = nc.dram_tensor("xt", [P, N * KT], dt_proj, kind="ExternalInput").ap()
    wc = nc.dram_tensor("wc", [P, KT * 3 * DL], dt_proj,
                        kind="ExternalInput").ap()
    o = nc.dram_tensor("o", [HL * (HD + 1), N], F32, kind="ExternalOutput").ap()

    with ExitStack() as ctx:
        pool = lambda name, bufs, **kw: ctx.enter_context(
            tc.tile_pool(name=name, bufs=bufs, **kw))
        const_pool = pool("const", 1)
        xpool = pool("x", 2 * NS)
        wpool = pool("w", 2)
        qk_pool = pool("qk", NS)
        kz_pool = pool("kz", HL * NS)
        vnat_pool = pool("vnat", 1)
        ppool = pool("p", 6)
        osb_pool = pool("osb", 3)
        ps = pool("ps", 2, space="PSUM")        # [128,1024] sc pair tiles: 2x2 banks
        ps_pj = pool("ps_pj", 2, space="PSUM")  # [128,512] proj/transpose: 2x1 bank
        ps_av = pool("ps_av", 2, space="PSUM")  # [65,512] accumulators: 2x1 bank

        ident = const_pool.tile([P, P], F32)
        make_identity(nc, ident[:])
        ident_r = const_pool.tile([P, P], dt_proj)
        nc.vector.tensor_copy(ident_r[:], ident[:])
        # multiplicative causal mask for [key-partition, query-col] diag blocks:
        # 1 where key <= query, 0 where key > query (bf16, post-exp multiply)
        tri16 = const_pool.tile([P, P], dt_p)
        nc.gpsimd.memset(tri16[:], 0.0)
        nc.gpsimd.affine_select(
            out=tri16[:], in_=tri16[:], compare_op=mybir.AluOpType.is_gt,
            fill=1.0, base=0, pattern=[[-1, P]], channel_multiplier=1)
        ones32 = const_pool.tile([P, 2 * NT], F32)
        nc.gpsimd.memset(ones32[:], 1.0)
        zeros = const_pool.tile([P, SPAN], F32)
        nc.gpsimd.memset(zeros[:], 0.0)
        zeros_r = const_pool.tile([P, SPAN], dt_proj)
        nc.vector.tensor_copy(zeros_r[:], zeros[:])

        # ---- DMA inputs: w halves + 8 x chunks (span, kt-half) ----
        x_tiles = [xpool.tile([P, KH * SPAN], dt_proj, tag="x", name=f"x{i}")
                   for i in range(2 * NS)]

        def x_slice(ns, kt):
            t = x_tiles[2 * ns + kt // KH]
            b = (kt % KH) * SPAN
            return t[:, b:b + SPAN]

        def dma_x(ns, half):
            w = KH * SPAN
            i = 2 * ns + half
            nc.sync.dma_start(x_tiles[i][:], xt[:, i * w:(i + 1) * w])

        # weights packed per m-chunk: wc columns = (chunk, kt, m) so chunk 0
        # only gates on its own 3KB slice (ready ~11us) and the x halves
        CH_W = (P, P, P, P, HD)
        CH_OFF = [sum(CH_W[:i]) for i in range(len(CH_W) + 1)]
        w_ch = [wpool.tile([P, KT * CH_W[ci]], dt_proj, tag=f"w{ci}",
                           name=f"w{ci}") for ci in range(5)]

        def w_slice(ci, kt):
            msz = CH_W[ci]
            return w_ch[ci][:, kt * msz:(kt + 1) * msz]

        def dma_w(ci):
            b = KT * CH_OFF[ci]
            w = KT * CH_W[ci]
            nc.sync.dma_start(w_ch[ci][:], wc[:, b:b + w])

        dma_w(0)
        dma_x(0, 0)
        dma_x(0, 1)
        for ci in range(1, 5):
            dma_w(ci)
        for ns in range(1, NS):
            dma_x(ns, 0)
            dma_x(ns, 1)

        # ---- per-span q/k/v tiles (zero-padded K=128 layout) ----
        qT01 = [qk_pool.tile([P, SPAN], dt_proj, tag="q01", name=f"q01_{i}") for i in range(NS)]
        qT2z = [qk_pool.tile([P, SPAN], dt_proj, tag="q2z", name=f"q2z_{i}") for i in range(NS)]
        vT01 = [qk_pool.tile([P, SPAN], dt_proj, tag="v01", name=f"v01_{i}") for i in range(NS)]
        vT2z = [qk_pool.tile([P, SPAN], dt_proj, tag="v2z", name=f"v2z_{i}") for i in range(NS)]
        kTz = [[kz_pool.tile([P, SPAN], dt_proj, tag="kz", name=f"kz_{h}_{i}")
                for i in range(NS)] for h in range(HL)]

        def zfill(ap):
            nc.vector.tensor_copy(ap, zeros[0:ap.shape[0], 0:ap.shape[1]])

        for ns in range(NS):
            zfill(qT2z[ns][HD:P, :])
            zfill(vT2z[ns][0:HD, :])
            zfill(kTz[0][ns][HD:P, :])
            zfill(kTz[1][ns][0:HD, :])
            zfill(kTz[2][ns][HD:P, :])

        # v natural layout: heads 0,1 interleaved per j-tile [v0|1|v1|1], head 2
        # separate [v2|1]; the ones column accumulates the softmax denominator.
        v_nat01 = vnat_pool.tile([P, NT * 2 * (HD + 1)], dt_p, tag="vnat01")
        v_nat2 = vnat_pool.tile([P, NT * (HD + 1)], dt_p, tag="vnat2")
        c01 = v_nat01[:].rearrange("p (t c) -> p t c", c=HD + 1)[:, :, HD]
        c2 = v_nat2[:].rearrange("p (t c) -> p t c", c=HD + 1)[:, :, HD]
        nc.vector.tensor_copy(c01, ones32[:])
        nc.vector.tensor_copy(c2, ones32[:, 0:NT])

        def vnat(h, jt):
            if h < 2:
                b = jt * 2 * (HD + 1) + h * (HD + 1)
                return v_nat01[:, b:b + HD + 1]
            b = jt * (HD + 1)
            return v_nat2[:, b:b + HD + 1]

        # ---- warmup: keep the PE busy while the first DMAs land ----
        warm = ps_pj.tile([P, SPAN], F32, tag="ps_pj", name="warm")

        def warmup(n):
            for _ in range(n):
                nc.tensor.matmul(warm[:], ident_r[:], zeros_r[:],
                                 start=True, stop=True)

        warmup(WARMUP_N)

        # ---- projections as an op list (5 chunk ops + 8 transpose ops) ----
        m_chunks = ((0, P, "q01"), (1, P, "k01"), (2, P, "v01"),
                    (3, P, "k2v2"), (4, HD, "q2"))

        def chunk_op(ns, ci, msz, what, midfill=0):
            pt = ps_pj.tile([msz, SPAN], F32, tag="ps_pj", name=f"pj_{ns}_{what}")
            for kt in range(KT):
                if midfill and kt == KH:
                    warmup(midfill)
                nc.tensor.matmul(
                    pt[:], w_slice(ci, kt), x_slice(ns, kt),
                    start=(kt == 0), stop=(kt == KT - 1))
            if what == "q01":
                nc.vector.tensor_copy(qT01[ns][:], pt[:])
            elif what == "k01":
                nc.vector.tensor_copy(kTz[0][ns][0:HD, :], pt[0:HD, :])
                nc.vector.tensor_copy(kTz[1][ns][HD:P, :], pt[HD:P, :])
            elif what == "v01":
                nc.vector.tensor_copy(vT01[ns][:], pt[:])
            elif what == "k2v2":
                nc.vector.tensor_copy(kTz[2][ns][0:HD, :], pt[0:HD, :])
                nc.vector.tensor_copy(vT2z[ns][HD:P, :], pt[HD:P, :])
            else:
                nc.vector.tensor_copy(qT2z[ns][0:HD, :], pt[:])

        def transp01_op(ns, c):
            jt = ns * CPS + c
            tp = ps_pj.tile([P, P], dt_proj, tag="ps_pj", name=f"tp_{jt}")
            nc.tensor.transpose(tp[:], vT01[ns][:, c * P:(c + 1) * P],
                                ident_r[:])
            nc.vector.tensor_copy(
                v_nat01[:].rearrange("p (t c) -> p t c", c=HD + 1)[
                    :, 2 * jt:2 * jt + 2, 0:HD],
                tp[:].rearrange("p (t c) -> p t c", c=HD))

        def transp2_op(ns, c):
            jt = ns * CPS + c
            tp2 = ps_pj.tile([P, P], dt_proj, tag="ps_pj", name=f"tp2_{jt}")
            nc.tensor.transpose(tp2[:], vT2z[ns][:, c * P:(c + 1) * P],
                                ident_r[:])
            nc.vector.tensor_copy(
                v_nat2[:, jt * (HD + 1):jt * (HD + 1) + HD], tp2[:, HD:P])

        def proj_ops(ns):
            ops = [lambda a=ci, b=msz, w=what: chunk_op(ns, a, b, w)
                   for (ci, msz, what) in m_chunks]
            for c in range(CPS):
                ops.append(lambda c=c: transp01_op(ns, c))
                ops.append(lambda c=c: transp2_op(ns, c))
            return ops

        pending = []
        pavs_left = [1]

        def drain_even():
            # spread pending ops evenly over the remaining insertion points
            if not pending:
                return False
            k = -(-len(pending) // max(pavs_left[0], 1))
            for _ in range(k):
                if pending:
                    pending.pop(0)()
            return True

        def finalize(s, h, av):
            ob = osb_pool.tile([HD + 1, SPAN], F32, tag="osb", name=f"ob{s}_{h}")
            if s == NS - 1 and h == HL - 1:
                # the very last evacuation rides ACT (idle after the final exp,
                # closer to PSUM) instead of queuing behind the DVE backlog
                nc.scalar.copy(ob[:], av[:])
            else:
                nc.vector.tensor_copy(ob[:], av[:])
            nc.sync.dma_start(
                o[h * (HD + 1):(h + 1) * (HD + 1),
                  s * SPAN:(s + 1) * SPAN], ob[:])

        # ---- attention: heads 0,1 fused pair loop; head 2 solo ----
        def attn01(s):
            njt = CPS * (s + 1)
            av0 = ps_av.tile([HD + 1, SPAN], F32, tag="ps_av", name=f"av0_{s}")
            av1 = ps_av.tile([HD + 1, SPAN], F32, tag="ps_av", name=f"av1_{s}")
            live = {}

            def emit_sc(jt):
                c_d = jt - CPS * s
                n0 = max(c_d, 0) * P
                ns_k, ck = jt // CPS, jt % CPS
                sc = ps.tile([P, 2 * SPAN], F32, tag="ps", name=f"sc01_{s}_{jt}")
                nc.tensor.matmul(sc[:, n0:SPAN],
                                 kTz[0][ns_k][:, ck * P:(ck + 1) * P],
                                 qT01[s][:, n0:SPAN], start=True, stop=True)
                nc.tensor.matmul(sc[:, SPAN + n0:2 * SPAN],
                                 kTz[1][ns_k][:, ck * P:(ck + 1) * P],
                                 qT01[s][:, n0:SPAN], start=True, stop=True)
                live[jt] = (sc, n0, c_d >= 0)

            def emit_pav(jt):
                sc, n0, diag = live.pop(jt)
                p = ppool.tile([P, 2 * SPAN], dt_p, tag="p", name=f"p01_{s}_{jt}")
                sc3 = sc[:].rearrange("q (t c) -> q t c", c=SPAN)
                p3 = p[:].rearrange("q (t c) -> q t c", c=SPAN)
                nc.scalar.activation(p3[:, :, n0:SPAN], sc3[:, :, n0:SPAN], EXP)
                if diag:
                    nc.vector.tensor_mul(p[:, n0:n0 + P], p[:, n0:n0 + P],
                                         tri16[:])
                    nc.vector.tensor_mul(
                        p[:, SPAN + n0:SPAN + n0 + P],
                        p[:, SPAN + n0:SPAN + n0 + P], tri16[:])
                # safe insertion point: every live sc tile's reader is emitted;
                # proj bursts and the two-ahead sc keep Tensor fed through the
                # exp latency
                drain_even()
                if jt + 2 < njt and jt + 2 not in live:
                    emit_sc(jt + 2)
                pavs_left[0] -= 1
                st, sp = (jt == 0), (jt == njt - 1)
                nc.tensor.matmul(av0[:, n0:SPAN], vnat(0, jt), p[:, n0:SPAN],
                                 start=st, stop=sp)
                nc.tensor.matmul(av1[:, n0:SPAN], vnat(1, jt),
                                 p[:, SPAN + n0:2 * SPAN], start=st, stop=sp)

            emit_sc(0)
            if njt > 1:
                emit_sc(1)
            for jt in range(njt):
                emit_pav(jt)
            finalize(s, 0, av0)
            finalize(s, 1, av1)

        def attn2(s):
            njt = CPS * (s + 1)
            av2 = ps_av.tile([HD + 1, SPAN], F32, tag="ps_av", name=f"av2_{s}")
            live = {}

            def emit_sc(jt):
                c_d = jt - CPS * s
                n0 = max(c_d, 0) * P
                ns_k, ck = jt // CPS, jt % CPS
                sc = ps.tile([P, 2 * SPAN], F32, tag="ps", name=f"sc2_{s}_{jt}")
                nc.tensor.matmul(sc[:, n0:SPAN],
                                 kTz[2][ns_k][:, ck * P:(ck + 1) * P],
                                 qT2z[s][:, n0:SPAN], start=True, stop=True)
                live[jt] = (sc, n0, c_d >= 0)

            def emit_pav(jt):
                sc, n0, diag = live.pop(jt)
                p = ppool.tile([P, 2 * SPAN], dt_p, tag="p", name=f"p2_{s}_{jt}")
                nc.scalar.activation(p[:, n0:SPAN], sc[:, n0:SPAN], EXP)
                if diag:
                    nc.vector.tensor_mul(p[:, n0:n0 + P], p[:, n0:n0 + P],
                                         tri16[:])
                drain_even()
                if jt + 2 < njt and jt + 2 not in live:
                    emit_sc(jt + 2)
                pavs_left[0] -= 1
                nc.tensor.matmul(av2[:, n0:SPAN], vnat(2, jt), p[:, n0:SPAN],
                                 start=(jt == 0), stop=(jt == njt - 1))

            emit_sc(0)
            if njt > 1:
                emit_sc(1)
            for jt in range(njt):
                emit_pav(jt)
            finalize(s, 2, av2)

        # spans 0 and 1 project standalone (warmup mid-fill covers the
        # kt0-2 -> kt3-5 x-DMA boundary of the first chunk); spans 2 and 3
        # interleave into the attention streams of spans 0 and 1, which are
        # small and latency-chained — the proj bursts keep the PE duty cycle
        # high so the HAM clock stays at 2.4 GHz
        for ns0 in range(2):
            first = ns0 == 0
            for (ci, msz, what) in m_chunks:
                chunk_op(ns0, ci, msz, what,
                         midfill=WARMUP_MID if first else 0)
                first = False
            for c in range(CPS):
                transp01_op(ns0, c)
                transp2_op(ns0, c)
        for s in range(NS):
            pending.extend(proj_ops(s + 2) if s + 2 < NS else [])
            pavs_left[0] = 2 * CPS * (s + 1)
            attn01(s)
            attn2(s)
            while pending:
                pending.pop(0)()


_NC_CACHE = {}


def _get_module(dt_proj=DT_PROJ, dt_p=DT_P):
    key = (dt_proj, dt_p)
    if key not in _NC_CACHE:
        nc = bacc.Bacc("TRN2", target_bir_lowering=False, debug=False)
        with tile.TileContext(nc) as tc:
            _build(nc, tc, dt_proj, dt_p)
        nc.compile()
        _NC_CACHE[key] = nc
    return _NC_CACHE[key]


def _in_maps(x, Wq, Wk, Wv):
    maps = []
    xT = [np.ascontiguousarray(
        x[b].T.reshape(KT, P, NS, SPAN).transpose(1, 2, 0, 3).reshape(P, -1))
        for b in range(B)]
    WqT, WkT, WvT = Wq.T, Wk.T, Wv.T
    for c in range(N_CORES):
        bc, g = divmod(c, N_CORES // B)
        s0 = g * DL
        wcomb = np.concatenate([
            WqT[:, s0:s0 + P], WkT[:, s0:s0 + P], WvT[:, s0:s0 + P],
            WkT[:, s0 + P:s0 + DL], WvT[:, s0 + P:s0 + DL],
            WqT[:, s0 + P:s0 + DL]], axis=1)
        # pack as (chunk, kt, m): per m-chunk, kt-major
        w3 = wcomb.reshape(KT, P, 3 * DL).transpose(1, 0, 2)  # [P, kt, m]
        parts = []
        for c0, c1 in ((0, P), (P, 2 * P), (2 * P, 3 * P), (3 * P, 4 * P),
                       (4 * P, 4 * P + HD)):
            parts.append(w3[:, :, c0:c1].reshape(P, -1))
        wpk = np.ascontiguousarray(np.concatenate(parts, axis=1))
        maps.append({
            "xt": xT[bc],
            "wc": wpk,
        })
    return maps


def kernel(x, Wq, Wk, Wv, _trace=False, _tmpdir=None, **_kw):
    x = np.asarray(x, dtype=np.float32)
    Wq = np.asarray(Wq, dtype=np.float32)
    Wk = np.asarray(Wk, dtype=np.float32)
    Wv = np.asarray(Wv, dtype=np.float32)
    assert x.shape == (B, N, D) and Wq.shape == (D, D)

    nc = _get_module()
    res = bass_utils.run_bass_kernel_spmd(
        nc, _in_maps(x, Wq, Wk, Wv), core_ids=list(range(N_CORES)),
        trace=_trace, tmpdir=_tmpdir)
    out = np.empty((B, N, D), np.float32)
    for c in range(N_CORES):
        bc, g = divmod(c, N_CORES // B)
        oT = res.results[c]["o"].astype(np.float64)
        for h in range(HL):
            blk = oT[h * (HD + 1):h * (HD + 1) + HD, :]
            den = oT[h * (HD + 1) + HD, :]
            out[bc, :, g * DL + h * HD:g * DL + (h + 1) * HD] = \
                (blk / den).T.astype(np.float32)
    if _trace:
        return out, res
    return out
